# revision 16
# baseline (speedup 1.0000x reference)
"""Trainium2 Bass kernel for nn_MultiHeadAttention (B=2, S=2048, DM=1024, H=8).

Sharding: data-parallel on batch x tensor-parallel on heads.
Core c in 0..7 handles batch b = c//4 and heads {2*(c%4), 2*(c%4)+1}.

v4: S^T-oriented dataflow (scores come out [key, query], so PV and the
rowsum-by-ones matmuls contract over the PSUM partition dim) on an fp16
datapath (same PE rate as bf16, 8x less quantization noise; fp8 was
measured to cost 2-3% output error per tensor - over tolerance).
Structural improvements over the v2 baseline:
  - attention starts right after the q/k projections: u0's scores/exps
    run while the V projection + transposes still stream on the PE
  - softmax r-chain: reciprocal_approx_fast straight off the rowsum PSUM
    row, then a small DRAM round-trip broadcast on idle DMA rings; the
    per-unit normalization is fused into the outp PSUM evacuation (one
    DVE tensor_tensor: outp * rbc -> fp16 outn)
  - masks ship as fp8 (exact 0/1 values, half the DMA bytes); mask-muls
    split DVE/gpsimd; all PSUM evacuations on DVE/ACT (gpsimd can't
    read PSUM)
  - out-projection tiles interleave into u2/u3's pair loops as their
    inputs become ready; final tiles' evacs split ACT/DVE; fp16 output
    partials summed on the host (bo added there too)
  - V transposes batch 4 per PSUM tile -> one 512-wide evacuation
"""

import sys

sys.path.insert(0, "/opt/trn_rl_repo")

import numpy as np
import ml_dtypes

import concourse.bass as bass
import concourse.tile as tile
from concourse import bacc, mybir
from concourse.bass import ts, ds
from concourse.bass_utils import run_bass_kernel_spmd

BF16 = mybir.dt.bfloat16
F32 = mybir.dt.float32
F16 = mybir.dt.float16
FP8 = mybir.dt.float8e4
Exp = mybir.ActivationFunctionType.Exp
Ident = mybir.ActivationFunctionType.Identity
MUL = mybir.AluOpType.mult
ADD = mybir.AluOpType.add

B, S, DM, H, DOUT = 2, 2048, 1024, 8, 1024
D = DM // H            # 128 head dim
NH = 2                 # heads per core
KP = 4                 # dm pair-chunks (2 x 128 contraction rows each)
OC = S // 128          # 16 key chunks
OP = OC // 2           # 8 key pair-chunks
NT = 512               # PSUM-bank-sized free tile (fp32)
HQ = 1024              # queries per half
SCALE = float(1.0 / np.sqrt(np.float32(D)))

# mask-mul engine split: oc%8 in set -> DVE, else gpsimd
DVE_SET = {0, 1, 3, 5, 6}


def build():
    nc = bacc.Bacc(None, target_bir_lowering=False)

    xT = nc.dram_tensor("xT", [3, KP, 128, 2, S], F16, kind="ExternalInput")
    maskT = nc.dram_tensor("maskT", [S, S], FP8, kind="ExternalInput")
    w2 = nc.dram_tensor("w2", [128, 3, KP, 2, NH, D], F16, kind="ExternalInput")
    b2 = nc.dram_tensor("b2", [128, 3, NH], F32, kind="ExternalInput")
    wo2 = nc.dram_tensor("wo2", [D, NH, DOUT], F16, kind="ExternalInput")
    ident = nc.dram_tensor("ident", [128, 128], F16, kind="ExternalInput")
    outT = nc.dram_tensor("outT", [DOUT, S], F16, kind="ExternalOutput")

    xq3 = [nc.sync, nc.gpsimd, nc.scalar]

    with tile.TileContext(nc) as tc:
        with (
            tc.tile_pool(name="const", bufs=1) as constp,
            tc.tile_pool(name="xin", bufs=12) as xp,
            tc.tile_pool(name="maskp", bufs=32) as mp,
            tc.tile_pool(name="rwork", bufs=1) as rbp,
            tc.tile_pool(name="fout", bufs=4) as fop,
            tc.tile_pool(name="psum", bufs=2, space="PSUM") as psp,
            tc.tile_pool(name="dram", bufs=2, space="DRAM") as dramp,
        ):
            # ---- small constants + exp table preload ----
            ones_col = constp.tile([128, 1], F16)
            nc.vector.memset(ones_col, 1.0)
            dummy2 = constp.tile([128, 1], BF16)
            nc.scalar.activation(out=dummy2, in_=ones_col, func=Exp, bias=0.0, scale=1.0)
            b_sb = constp.tile([128, 3, NH], F32)
            nc.gpsimd.dma_start(out=b_sb, in_=b2[:])

            # ---- weights early, then x chunk halves in consumption order ----
            w_sb = constp.tile([128, 3, KP, 2, NH, D], F16)
            for c in range(KP):
                nc.scalar.dma_start(out=w_sb[:, 0, c, :, :, :], in_=w2[:, 0, c, :, :, :])
            ident_sb = constp.tile([128, 128], F16)
            xts = []  # [t][c] -> [128, 2, S]
            for t in range(3):
                xts.append(
                    [xp.tile([128, 2, S], F16, tag="x", name=f"x{t}_{c}") for c in range(KP)]
                )
            qi = 0
            # q cb0, k cb0, k cb1, q cb1
            for t, cb in ((0, 0), (1, 0), (1, 1), (0, 1)):
                for c in range(KP):
                    xq3[qi % 3].dma_start(
                        out=xts[t][c][:, :, ts(cb, HQ)],
                        in_=xT[t, c, :, :, ts(cb, HQ)],
                    )
                    qi += 1
                if t == 1 and cb == 0:
                    nc.gpsimd.dma_start(
                        out=w_sb[:, 1, :, :, :, :], in_=w2[:, 1, :, :, :, :]
                    )
                    nc.scalar.dma_start(out=ident_sb, in_=ident[:])
                if t == 1 and cb == 1:
                    nc.gpsimd.dma_start(
                        out=w_sb[:, 2, :, :, :, :], in_=w2[:, 2, :, :, :, :]
                    )

            mtile = {
                (half, oc): mp.tile([128, HQ], FP8, tag="mask", name=f"m{half}_{oc}")
                for half in range(2)
                for oc in range(OC)
            }
            wo_sb = constp.tile([D, NH, DOUT], F16)

            def issue_late_dmas():
                # v chunks (split rings, cb0 first), then mask half0, half1
                xq2 = [nc.sync, nc.gpsimd]
                for cb in range(2):
                    for c in range(KP):
                        xq2[c % 2].dma_start(
                            out=xts[2][c][:, :, ts(cb, HQ)],
                            in_=xT[2, c, :, :, ts(cb, HQ)],
                        )
                for half in range(2):
                    for oc in range(OC):
                        xq2[oc % 2].dma_start(
                            out=mtile[(half, oc)],
                            in_=maskT[
                                oc * 128 : (oc + 1) * 128, half * HQ : (half + 1) * HQ
                            ],
                        )
                    if half == 0:
                        nc.scalar.dma_start(out=wo_sb, in_=wo2[:])

            qk_sb = constp.tile([128, 2, NH, S], F16)
            vpt_sb = xp.tile([128, NH, S], F16, tag="x", name="vpt")
            vp_sb = constp.tile([128, OP, 2, NH, D], F16)
            outn_sb = constp.tile([128, NH, S], F16)
            rbc = {}  # u -> [j] broadcast f32 tiles [128, NT]

            # ---- projections: two [128, 2*NT] psum accs (tag "s") ----
            def proj_pass(t, cb, evac):
                acc = [
                    psp.tile([128, 2 * NT], F32, tag="s", name=f"acc{t}_{cb}_{h}")
                    for h in range(NH)
                ]
                for c in range(KP):
                    for j in range(2):
                        for h in range(NH):
                            for i2 in range(2):
                                nc.tensor.matmul(
                                    acc[h][:, ts(i2, NT)],
                                    w_sb[:, t, c, j, h, :],
                                    xts[t][c][:, j, ds(cb * HQ + i2 * NT, NT)],
                                    start=(c == 0 and j == 0),
                                    stop=(c == KP - 1 and j == 1),
                                )
                for h in range(NH):
                    for i2 in range(2):
                        evac(h, cb * HQ + i2 * NT, acc[h][:, ts(i2, NT)])

            def qk_evac(t):
                def e(h, off, accap):
                    # ACT is idle pre-attention
                    nc.scalar.activation(
                        out=qk_sb[:, t, h, ds(off, NT)],
                        in_=accap,
                        func=Ident,
                        bias=b_sb[:, t, h : h + 1],
                        scale=1.0,
                    )
                return e

            def v_evac(h, off, accap):
                # gpsimd cannot read PSUM; DVE owns this one
                nc.vector.tensor_scalar_add(
                    out=vpt_sb[:, h, ds(off, NT)],
                    in0=accap,
                    scalar1=b_sb[:, 2, h : h + 1],
                )

            def v_transposes_pair(c):
                # 4 transposes (j, h) into one PSUM tile, then a single
                # batched 512-wide evacuation into the PV lhsT layout
                tps = psp.tile([128, 4 * D], F16, tag="facc", bufs=1, name=f"tps{c}")
                for j in range(2):
                    for h in range(NH):
                        nc.tensor.transpose(
                            tps[:, ds((j * NH + h) * D, D)],
                            vpt_sb[:, h, ds((2 * c + j) * 128, 128)],
                            ident_sb,
                        )
                nc.vector.tensor_copy(vp_sb[:, c, :, :, :], tps)

            # ---- attention units: (half, head) ----
            pending = []  # deferred epilogue thunks from previous units

            def make_unit(u, half, hh):
                sps = {}
                ets = {}
                pm2 = {}
                done = set()
                i0 = half * HQ

                def scores(oc):
                    if ("s", oc) in done:
                        return
                    done.add(("s", oc))
                    t_ = psp.tile([128, 2 * NT], F32, tag="s", name=f"sps{u}_{oc}")
                    for j in range(2):
                        nc.tensor.matmul(
                            t_[:, ts(j, NT)],
                            qk_sb[:, 1, hh, ds(oc * 128, 128)],
                            qk_sb[:, 0, hh, ds(i0 + j * NT, NT)],
                            start=True,
                            stop=True,
                        )
                    sps[oc] = t_

                def expo(oc):
                    if ("e", oc) in done:
                        return
                    done.add(("e", oc))
                    e_ = xp.tile([128, 2 * NT], F16, tag="x", name=f"e{u}_{oc}")
                    nc.scalar.activation(
                        out=e_, in_=sps.pop(oc), func=Exp, bias=0.0, scale=SCALE
                    )
                    ets[oc] = e_

                def maskmul(oc):
                    if ("m", oc) in done:
                        return
                    done.add(("m", oc))
                    c = oc // 2
                    if c not in pm2:
                        pm2[c] = xp.tile(
                            [128, 2, 2 * NT], F16, tag="x", name=f"pm{u}_{c}"
                        )
                    eng = nc.vector if (oc % 8) in DVE_SET else nc.gpsimd
                    eng.tensor_mul(pm2[c][:, oc % 2, :], ets.pop(oc), mtile[(half, oc)])

                return scores, expo, maskmul, pm2, (half, hh)

            def do_unit(u, state, prefix_ocs=(), interleave=None, next_state=None):
                scores, expo, maskmul, pm2, (half, hh) = state
                i0 = half * HQ
                # previous unit's norms first: they release the shared outp
                # buffers (their rbc broadcasts landed during that unit's tail)
                while pending:
                    pending.pop(0)()
                # masks for prefix ocs were deferred to keep the DVE queue
                # clear for the V-phase / previous-unit work
                for oc in prefix_ocs:
                    maskmul(oc)
                for oc in range(4):
                    scores(oc)
                    expo(oc)
                    maskmul(oc)
                outp = [
                    psp.tile([128, NT], F32, tag="acc", name=f"outp{u}_{j}")
                    for j in range(2)
                ]
                rp = psp.tile([128, NT], F32, tag="rp", bufs=1, name=f"rp{u}")
                for c in range(OP):
                    if c + 2 < OP:
                        scores(2 * c + 4)
                        scores(2 * c + 5)
                    pm = pm2.pop(c)
                    for jk in range(2):
                        for j in range(2):
                            nc.tensor.matmul(
                                outp[j],
                                vp_sb[:, c, jk, hh, :],
                                pm[:, jk, ts(j, NT)],
                                start=(c == 0 and jk == 0),
                                stop=(c == OP - 1 and jk == 1),
                            )
                        for j in range(2):
                            nc.tensor.matmul(
                                rp[32 * j : 32 * j + 1, :],
                                ones_col,
                                pm[:, jk, ts(j, NT)],
                                start=(c == 0 and jk == 0),
                                stop=(c == OP - 1 and jk == 1),
                            )
                    if c + 2 < OP:
                        expo(2 * c + 4)
                        maskmul(2 * c + 4)
                        expo(2 * c + 5)
                        maskmul(2 * c + 5)
                    if next_state is not None and c >= 6:
                        # warm up the next unit's score/exp pipeline
                        oc = c - 6
                        next_state[0](oc)
                        next_state[1](oc)
                    if interleave is not None:
                        interleave(c)

                # epilogue thunks: evacuate the rowsum rows, DRAM round-trip
                # broadcast of r, reciprocal on the broadcast tile, then the
                # fused evac+normalize of outp
                r2 = rbp.tile([33, NT], F32, tag="rinv", name=f"r2_{u}")
                rd = dramp.tile([1, 2 * NT], F32, tag="rd", name=f"rd{u}")
                rbr = [
                    rbp.tile([128, NT], F32, tag=f"rbr{u % 2}_{j}", name=f"rbr{u}_{j}")
                    for j in range(2)
                ]
                rbc[u] = [
                    rbp.tile([128, NT], F32, tag=f"rbc{u % 2}_{j}", name=f"rbc{u}_{j}")
                    for j in range(2)
                ]
                rq = [nc.gpsimd, nc.sync]

                def t_recip():
                    for j in range(2):
                        nc.vector.tensor_copy(
                            r2[32 * j : 32 * j + 1, :], rp[32 * j : 32 * j + 1, :]
                        )
                    for j in range(2):
                        rq[j].dma_start(
                            out=rd[:, ts(j, NT)], in_=r2[32 * j : 32 * j + 1, :]
                        )
                    for j in range(2):
                        rq[j].dma_start(
                            out=rbr[j],
                            in_=rd[:, ts(j, NT)].to_broadcast([128, NT]),
                        )

                def t_norm(j):
                    # reciprocal on the full-partition broadcast (the proven
                    # custom-DVE shape), then fused evac+normalize -> fp16
                    nc.vector.reciprocal_approx_fast(out=rbc[u][j], in_=rbr[j])
                    nc.vector.tensor_mul(
                        outn_sb[:, hh, ds(i0 + j * NT, NT)], outp[j], rbc[u][j]
                    )

                # recip immediately (clears the rp WAR before the next unit's
                # first rowsum); only the norms are deferred
                t_recip()
                return [lambda: t_norm(0), lambda: t_norm(1)]

            # ---- out-projection windows ----
            outq = [nc.sync, nc.gpsimd, nc.scalar]
            wq_n = [0]

            def win_tile(half, n, tail=False):
                # n in 0..15 -> (dc, itl): itl fastest so weights amortize
                dc, itl = n // 2, n % 2
                i0 = half * HQ
                facc = psp.tile(
                    [128, NT], F32, tag="facc", bufs=1, name=f"facc{half}_{itl}_{dc}"
                )
                for hh in range(NH):
                    nc.tensor.matmul(
                        facc,
                        wo_sb[:, hh, ds(dc * 128, 128)],
                        outn_sb[:, hh, ds(i0 + itl * NT, NT)],
                        start=(hh == 0),
                        stop=(hh == NH - 1),
                    )
                fsb = fop.tile([128, NT], F16, tag="f", name=f"f{half}_{itl}_{dc}")
                if tail and n % 2 == 1:
                    # ACT is only free once the exps are done
                    nc.scalar.activation(
                        out=fsb, in_=facc, func=Ident, bias=0.0, scale=1.0
                    )
                else:
                    nc.vector.tensor_copy(fsb, facc)
                outq[wq_n[0] % 3].dma_start(
                    out=outT[ds(dc * 128, 128), ds(i0 + itl * NT, NT)], in_=fsb
                )
                wq_n[0] += 1

            def win_pair(half, dc):
                # tail-only: both itl tiles of one dc slab in a 2-bank psum,
                # evacuated by DVE and ACT in parallel, one 256KB DMA out
                i0 = half * HQ
                facc2 = psp.tile([128, 2 * NT], F32, tag="s", name=f"fp{half}_{dc}")
                for itl in range(2):
                    for hh in range(NH):
                        nc.tensor.matmul(
                            facc2[:, ts(itl, NT)],
                            wo_sb[:, hh, ds(dc * 128, 128)],
                            outn_sb[:, hh, ds(i0 + itl * NT, NT)],
                            start=(hh == 0),
                            stop=(hh == NH - 1),
                        )
                fsb2 = fop.tile([128, 2 * NT], F16, tag="f", name=f"fp2{half}_{dc}")
                nc.vector.tensor_copy(fsb2[:, ts(0, NT)], facc2[:, ts(0, NT)])
                nc.scalar.activation(
                    out=fsb2[:, ts(1, NT)],
                    in_=facc2[:, ts(1, NT)],
                    func=Ident,
                    bias=0.0,
                    scale=1.0,
                )
                outq[wq_n[0] % 3].dma_start(
                    out=outT[ds(dc * 128, 128), ds(i0, 2 * NT)], in_=fsb2
                )
                wq_n[0] += 1

            # ---- schedule ----
            proj_pass(0, 0, qk_evac(0))
            proj_pass(1, 0, qk_evac(1))
            proj_pass(1, 1, qk_evac(1))
            issue_late_dmas()
            proj_pass(0, 1, qk_evac(0))

            # V projection runs per (cb, head) on the "acc" psum pair so the
            # "s" bufs stay free for u0's score pipeline; u0 scores+exps fill
            # the PE/ACT while V-chunk DMAs arrive (keeps the HAM warm too)
            u0 = make_unit(0, 0, 0)
            u1 = make_unit(1, 0, 1)
            u2 = make_unit(2, 1, 0)
            u3 = make_unit(3, 1, 1)
            feed_n = [0]

            def u0_feed():
                if feed_n[0] < 7:
                    u0[0](feed_n[0])
                    u0[1](feed_n[0])
                    feed_n[0] += 1

            def vproj(cb, h):
                acc = [
                    psp.tile([128, NT], F32, tag="acc", name=f"vacc{cb}_{h}_{i2}")
                    for i2 in range(2)
                ]
                for c in range(KP):
                    for j in range(2):
                        for i2 in range(2):
                            nc.tensor.matmul(
                                acc[i2],
                                w_sb[:, 2, c, j, h, :],
                                xts[2][c][:, j, ds(cb * HQ + i2 * NT, NT)],
                                start=(c == 0 and j == 0),
                                stop=(c == KP - 1 and j == 1),
                            )
                    if c % 2 == 1:
                        u0_feed()
                for i2 in range(2):
                    v_evac(h, cb * HQ + i2 * NT, acc[i2])

            for cb in range(2):
                for h in range(NH):
                    vproj(cb, h)
            for c in range(OP):
                v_transposes_pair(c)
                if c % 2 == 1:
                    u0_feed()

            pending.extend(do_unit(0, u0, prefix_ocs=tuple(range(7)), next_state=u1))
            pending.extend(do_unit(1, u1, prefix_ocs=(0, 1), next_state=u2))
            # winA tiles become ready once u0+u1 norms land; interleave them
            # into u2/u3's pair loops
            pending.extend(
                do_unit(
                    2,
                    u2,
                    prefix_ocs=(0, 1),
                    interleave=lambda c: win_tile(0, c - 3) if c >= 3 else None,
                    next_state=u3,
                )
            )
            pending.extend(
                do_unit(
                    3,
                    u3,
                    prefix_ocs=(0, 1),
                    interleave=lambda c: win_tile(0, 5 + c),
                )
            )
            # leftover winA tiles keep the PE busy while u3's r-chain resolves
            for n in range(13, 16):
                win_tile(0, n, tail=True)
            for th in pending:
                th()
            pending.clear()
            for dc in range(8):
                win_pair(1, dc)

    return nc


_NC_CACHE = None


def _get_nc():
    global _NC_CACHE
    if _NC_CACHE is None:
        nc = build()
        nc.compile()
        _NC_CACHE = nc
    return _NC_CACHE


def make_in_maps(q, k, v, mask, Wq, bq, Wk, bk, Wv, bv, Wo, bo):
    e4 = ml_dtypes.float8_e4m3fn
    f16 = np.float16
    q = np.asarray(q, np.float32)
    k = np.asarray(k, np.float32)
    v = np.asarray(v, np.float32)
    mask = np.asarray(mask)
    Ws = [np.asarray(w, np.float32) for w in (Wq, Wk, Wv)]
    bs = [np.asarray(b_, np.float32) for b_ in (bq, bk, bv)]
    Wo = np.asarray(Wo, np.float32)

    # xT16[b][t, c, p, j, s] = x_t[b][s, (2c+j)*128+p]
    xT16, maskT8 = [], []
    for b in range(B):
        xs = np.stack([q[b].T, k[b].T, v[b].T])  # [3, DM, S]
        xT16.append(
            np.ascontiguousarray(
                xs.reshape(3, KP, 2, 128, S).transpose(0, 1, 3, 2, 4)
            ).astype(f16)
        )
        maskT8.append(np.ascontiguousarray(mask[b].T.astype(np.float32)).astype(e4))

    # w2[p, t, c, j, h, d] = W_t[(2c+j)*128+p, d*H + h0+h]
    Wr = [W.reshape(KP, 2, 128, D, H) for W in Ws]  # [c, j, p, d, hglob]
    br = [b_.reshape(D, H) for b_ in bs]
    ident = np.eye(128, dtype=np.float32).astype(f16)

    in_maps = []
    for core in range(8):
        b = core // 4
        h0 = NH * (core % 4)
        w_core = np.empty((128, 3, KP, 2, NH, D), np.float32)
        for t in range(3):
            for hi in range(NH):
                # Wr[t][c, j, p, d] for head h0+hi -> [p, c, j, d]
                w_core[:, t, :, :, hi, :] = Wr[t][:, :, :, :, h0 + hi].transpose(
                    2, 0, 1, 3
                )
        b_core = np.empty((128, 3, NH), np.float32)
        for t in range(3):
            for hi in range(NH):
                b_core[:, t, hi] = br[t][:, h0 + hi]
        wo_core = np.stack([Wo[h0 + hi :: H, :] for hi in range(NH)], axis=1)
        in_maps.append(
            {
                "xT": xT16[b],
                "ident": ident,
                "maskT": maskT8[b],
                "w2": np.ascontiguousarray(w_core).astype(f16),
                "b2": np.ascontiguousarray(b_core),
                "wo2": np.ascontiguousarray(wo_core).astype(f16),
            }
        )
    return in_maps


def unshard(results, bo=None):
    out = np.zeros((B, DOUT, S), np.float32)
    for c in range(8):
        out[c // 4] += np.asarray(results[c]["outT"], np.float32)
    out = np.ascontiguousarray(out.transpose(0, 2, 1))
    if bo is not None:
        out += np.asarray(bo, np.float32)
    return out


def kernel(**inputs):
    in_maps = make_in_maps(**inputs)
    nc = _get_nc()
    res = run_bass_kernel_spmd(nc, in_maps, core_ids=list(range(8)))
    return unshard(res.results, bo=inputs.get("bo"))
